# revision 1
# baseline (speedup 1.0000x reference)
"""Trainium2 Bass kernel for nn_Cross_Attention_Block_3624952397825.

Mathematical structure exploited: the reference takes ``out[:, -1, :]`` --
the attention output of the LAST query token. That token comes from the
zero row appended by ``jnp.pad`` AFTER the conv stack, so its query vector
is exactly zero, its attention scores are exactly zero, and softmax over
exact zeros is exactly uniform (1/4096).  Hence

    bins[b] = mean_k V[b, k, :] = (mean_k lidar[b, k, :]) @ wv
    out[b]  = MLP3(leaky_relu chain)(bins[b])

The conv block, Q/K projections, and softmax are structurally dead code
for ANY input values.  Additionally there is no nonlinearity between wv
and wo1, so W1 = wv @ wo1 [256, 128] is constant-folded on the host.

Per core (2 batches): stream lidar as fp16 [128, 4096] tiles (8 KiB per
partition -> full single-queue DMA rate), reduce the 4096 points with
ones^T @ tile matmuls on TensorE (fp16 x fp16 products are exact for a
1.0 stationary; accumulation is fp32 in PSUM), then a tiny fp16 MLP
(fp32 biases, fp32 final add).  Weights ride the second HWDGE queue
(ScalarE) so the lidar FIFO is never interrupted; batch 0 streams first
so its MLP overlaps batch 1's DMA.  Measured model error ~6e-4.
"""

import numpy as np

B, NPTS, CH, DM = 16, 4096, 256, 1024
N_CORES = 8
BL = B // N_CORES            # batches per core
P = 128
TILE_F = 4096                # free dim of lidar tiles (16 pts x 256 ch)
N_TILES = NPTS * CH // (P * TILE_F)   # 2 tiles per batch

# fp16 weight pack layout (free dim)
OFF_W1 = 0                   # 2 k-chunks x 128   (W1 = wv @ wo1)
OFF_WO2 = 256                # 128
OFF_WO3 = 384                # 256  (stored [K=128, 256] for row-form output)
OFF_ONE16 = 640              # fp16 ones column
W16_F = 641
# fp32 pack columns
C_B1, C_B2 = 0, 1
W32_F = 4

_CACHE = {}


def _build_program():
    import concourse.bacc as bacc
    import concourse.mybir as mybir
    from concourse.tile import TileContext

    f32 = mybir.dt.float32
    f16 = mybir.dt.float16
    Alu = mybir.AluOpType
    Act = mybir.ActivationFunctionType

    nc = bacc.Bacc("TRN2")
    lidar = nc.dram_tensor("lidar16", [BL, NPTS, CH], f16, kind="ExternalInput")
    wp16d = nc.dram_tensor("wp16", [P, W16_F], f16, kind="ExternalInput")
    wp32d = nc.dram_tensor("wp32", [P, W32_F], f32, kind="ExternalInput")
    b3rowd = nc.dram_tensor("b3row", [1, CH], f32, kind="ExternalInput")
    out_rows = nc.dram_tensor("out_rows", [BL, CH], f32, kind="ExternalOutput")

    # [BL, 4096, 256] -> [(b t), 128, 4096]; 8 KiB contiguous per partition.
    lv = lidar[:, :, :].rearrange("b (t p q) c -> (b t) p (q c)", p=P, q=16)

    with TileContext(nc) as tc:
        with (
            tc.tile_pool(name="w", bufs=1) as wpool,
            tc.tile_pool(name="io", bufs=4) as iopool,
            tc.tile_pool(name="small", bufs=1) as spool,
            tc.tile_pool(name="ps", bufs=2, space="PSUM") as pspool,
            tc.tile_pool(name="orp", bufs=2, space="PSUM") as orpool,
            tc.tile_pool(name="mm", bufs=3, space="PSUM") as mmpool,
        ):
            # weights on the ScalarE HWDGE queue; lidar owns the SP queue
            wp16 = wpool.tile([P, W16_F], f16, tag="wp16")
            nc.scalar.dma_start(out=wp16[:, :], in_=wp16d[:, :])
            wp32 = wpool.tile([P, W32_F], f32, tag="wp32")
            nc.scalar.dma_start(out=wp32[:, :], in_=wp32d[:, :])
            b3row = wpool.tile([1, CH], f32, tag="b3row")
            nc.scalar.dma_start(out=b3row[:, :], in_=b3rowd[:, :])
            ones16 = wp16[:, OFF_ONE16:OFF_ONE16 + 1]

            for b in range(BL):
                # ---- point reduction: ones^T @ tile on TensorE ----
                # fp16 x 1.0 products are exact; fp32 PSUM accumulation.
                # 512-wide moving operand (2 points x 256 ch per matmul).
                MM_F = 2 * CH
                sred = pspool.tile([1, MM_F], f32, tag="sred")
                nmm = N_TILES * (TILE_F // MM_F)
                i = 0
                for t in range(N_TILES):
                    tin = iopool.tile([P, TILE_F], f16, tag="tin")
                    nc.sync.dma_start(out=tin[:, :], in_=lv[b * N_TILES + t, :, :])
                    for j in range(TILE_F // MM_F):
                        nc.tensor.matmul(sred[:, :], lhsT=ones16,
                                         rhs=tin[:, j * MM_F:(j + 1) * MM_F],
                                         start=(i == 0), stop=(i == nmm - 1))
                        i += 1
                # fold [1, 512] -> fp16 [1, 256] sums via SBUF bounce
                s512 = spool.tile([1, MM_F], f32, tag=f"s512{b}")
                nc.scalar.copy(out=s512[:, :], in_=sred[:, :])
                s16 = spool.tile([1, CH], f16, tag=f"s16{b}")
                nc.vector.tensor_add(out=s16[:, :], in0=s512[0:1, 0:CH],
                                     in1=s512[0:1, CH:MM_F])
                # transpose row [1, 256] -> 2 x [128, 1] via K=1 fp16 matmuls;
                # mean scale (1/4096, exact power of two) folded into the copy
                mt = []
                for k in range(2):
                    mtp = mmpool.tile([P, 1], f32, tag="mm")
                    nc.tensor.matmul(mtp[:, :], lhsT=s16[0:1, k * P:(k + 1) * P],
                                     rhs=ones16[0:1, 0:1], start=True, stop=True)
                    mt16 = spool.tile([P, 1], f16, tag=f"mt{b}{k}")
                    nc.scalar.activation(mt16[:, :], mtp[:, :], Act.Copy,
                                         scale=float(1.0 / NPTS))
                    mt.append(mt16)

                def leaky(zp, bias_col, tag):
                    z = spool.tile([P, 1], f16, tag=f"z{tag}")
                    nc.scalar.activation(z[:, :], zp[:, :], Act.Identity,
                                         bias=wp32[:, bias_col:bias_col + 1], scale=1.0)
                    h = spool.tile([P, 1], f16, tag=f"h{tag}")
                    nc.vector.scalar_tensor_tensor(out=h[:, :], in0=z[:, :], scalar=0.01,
                                                   in1=z[:, :], op0=Alu.mult, op1=Alu.max)
                    return h

                # h1 = leaky(m @ W1 + b1), W1 pre-folded on host
                h1p = mmpool.tile([P, 1], f32, tag="mm")
                for k in range(2):
                    nc.tensor.matmul(h1p[:, :],
                                     lhsT=wp16[:, OFF_W1 + k * P: OFF_W1 + (k + 1) * P],
                                     rhs=mt[k][:, :], start=(k == 0), stop=(k == 1))
                h1 = leaky(h1p, C_B1, f"1{b}")

                h2p = mmpool.tile([P, 1], f32, tag="mm")
                nc.tensor.matmul(h2p[:, :], lhsT=wp16[:, OFF_WO2:OFF_WO2 + P],
                                 rhs=h1[:, :], start=True, stop=True)
                h2 = leaky(h2p, C_B2, f"2{b}")

                # final layer in row form: h2^T @ wo3 -> [1, 256]
                orp = orpool.tile([1, CH], f32, tag="orp")
                nc.tensor.matmul(orp[:, :], lhsT=h2[:, :],
                                 rhs=wp16[:, OFF_WO3:OFF_WO3 + CH],
                                 start=True, stop=True)
                orow = spool.tile([1, CH], f32, tag=f"orow{b}")
                nc.vector.tensor_add(out=orow[:, :], in0=orp[:, :], in1=b3row[:, :])
                nc.scalar.dma_start(out=out_rows[b:b + 1, :], in_=orow[:, :])

    nc.compile()
    return nc


def _pack_weights(inputs):
    wv = np.asarray(inputs["wv"], np.float64)
    wo1 = np.asarray(inputs["wo1"], np.float64)
    W1 = (wv @ wo1)                           # [256, 128], no nonlinearity between

    wp16 = np.zeros((P, W16_F), np.float16)
    wp16[:, OFF_W1:OFF_W1 + P] = W1[0:128, :]
    wp16[:, OFF_W1 + P:OFF_W1 + 2 * P] = W1[128:256, :]
    wp16[:, OFF_WO2:OFF_WO2 + P] = np.asarray(inputs["wo2"], np.float32)
    wp16[:, OFF_WO3:OFF_WO3 + CH] = np.asarray(inputs["wo3"], np.float32)
    wp16[:, OFF_ONE16] = 1.0

    wp32 = np.zeros((P, W32_F), np.float32)
    wp32[:, C_B1] = np.asarray(inputs["b1"], np.float32)
    wp32[:, C_B2] = np.asarray(inputs["b2"], np.float32)
    b3row = np.asarray(inputs["b3"], np.float32).reshape(1, CH)
    return wp16, wp32, b3row


def kernel(**inputs):
    from concourse.bass_utils import run_bass_kernel_spmd

    if "nc" not in _CACHE:
        _CACHE["nc"] = _build_program()
    nc = _CACHE["nc"]

    lidar16 = np.ascontiguousarray(
        np.asarray(inputs["lidar"], dtype=np.float32).astype(np.float16))
    wp16, wp32, b3row = _pack_weights(inputs)

    in_maps = [
        {"lidar16": lidar16[i * BL:(i + 1) * BL], "wp16": wp16,
         "wp32": wp32, "b3row": b3row}
        for i in range(N_CORES)
    ]
    res = run_bass_kernel_spmd(nc, in_maps, list(range(N_CORES)),
                               **_CACHE.get("run_kwargs", {}))
    _CACHE["last_results"] = res
    out = np.concatenate([res.results[i]["out_rows"] for i in range(N_CORES)], axis=0)
    return np.ascontiguousarray(out, dtype=np.float32)



# revision 12
# speedup vs baseline: 1.1023x; 1.1023x over previous
"""Trainium2 Bass kernel for nn_Cross_Attention_Block_3624952397825.

Mathematical structure exploited: the reference takes ``out[:, -1, :]`` --
the attention output of the LAST query token. That token comes from the
zero row appended by ``jnp.pad`` AFTER the conv stack, so its query vector
is exactly zero, its attention scores are exactly zero, and softmax over
exact zeros is exactly uniform (1/4096).  Hence

    bins[b] = mean_k V[b, k, :] = (mean_k lidar[b, k, :]) @ wv
    out[b]  = MLP3(leaky_relu chain)(bins[b])

The conv block, Q/K projections, and softmax are structurally dead code
for ANY input values.  Additionally there is no nonlinearity between wv
and wo1, so W1 = wv @ wo1 [256, 128] is constant-folded on the host.

Per core (2 batches): stream lidar as fp16 [128, 4096] tiles (8 KiB per
partition -> full single-queue DMA rate), reduce the 4096 points with
one-hot^T @ tile matmuls on TensorE (fp16 x 1.0 products are exact;
fp32 PSUM accumulation).  Batch b's tiles use a one-hot stationary
column so its point-sum lands in PSUM ROW b of a shared [2, 512]
accumulator -- both batches then ride ONE post-stream dependency chain
(fold -> identity-matmul transpose -> tiny MLP on [128, 2] tiles ->
one [2, 256] output matmul), instead of two serial per-batch chains.
Weights ride the second HWDGE queue (ScalarE) so the lidar FIFO is
never interrupted.  Measured model error ~6e-4.
"""

import numpy as np

B, NPTS, CH, DM = 16, 4096, 256, 1024
N_CORES = 8
BL = B // N_CORES            # batches per core
P = 128
TILE_F = 4096                # free dim of lidar tiles (16 pts x 256 ch)
N_TILES = NPTS * CH // (P * TILE_F)   # 2 tiles per batch

# fp16 weight pack layout (free dim)
OFF_W1 = 0                   # 2 k-chunks x 128   (W1 = wv @ wo1)
OFF_WO2 = 256                # 128
OFF_WO3 = 384                # 256  (stored [K=128, 256] for row-form output)
OFF_EB = 640                 # 3 cols [1,0,1]: lhsT pair (b, b+1) is one-hot b
OFF_I2 = 643                 # [2, 2] identity (rows 0-1 only)
W16_F = 645
# fp32 pack columns
C_B1, C_B2 = 0, 1
W32_F = 4

_CACHE = {}


def _build_program():
    import concourse.bacc as bacc
    import concourse.mybir as mybir
    from concourse.tile import TileContext

    f32 = mybir.dt.float32
    f16 = mybir.dt.float16
    Alu = mybir.AluOpType
    Act = mybir.ActivationFunctionType

    nc = bacc.Bacc("TRN2")
    lidar = nc.dram_tensor("lidar16", [BL, NPTS, CH], f16, kind="ExternalInput")
    wp16d = nc.dram_tensor("wp16", [P, W16_F], f16, kind="ExternalInput")
    wp32d = nc.dram_tensor("wp32", [P, W32_F], f32, kind="ExternalInput")
    b3rowd = nc.dram_tensor("b3row2", [BL, CH], f32, kind="ExternalInput")
    out_rows = nc.dram_tensor("out_rows", [BL, CH], f32, kind="ExternalOutput")

    # [BL, 4096, 256] -> [(b t), 128, 4096]; 8 KiB contiguous per partition.
    lv = lidar[:, :, :].rearrange("b (t p q) c -> (b t) p (q c)", p=P, q=16)

    with TileContext(nc) as tc:
        with (
            tc.tile_pool(name="w", bufs=1) as wpool,
            tc.tile_pool(name="io", bufs=4) as iopool,
            tc.tile_pool(name="small", bufs=1) as spool,
            tc.tile_pool(name="ps", bufs=1, space="PSUM") as pspool,
            tc.tile_pool(name="mm", bufs=3, space="PSUM") as mmpool,
        ):
            # weights on the ScalarE HWDGE queue; lidar owns the SP queue
            wp16 = wpool.tile([P, W16_F], f16, tag="wp16")
            nc.scalar.dma_start(out=wp16[:, :], in_=wp16d[:, :])
            wp32 = wpool.tile([P, W32_F], f32, tag="wp32")
            nc.scalar.dma_start(out=wp32[:, :], in_=wp32d[:, :])
            b3row2 = wpool.tile([BL, CH], f32, tag="b3row2")
            nc.scalar.dma_start(out=b3row2[:, :], in_=b3rowd[:, :])

            # ---- point reduction: one-hot^T @ tile on TensorE ----
            # batch b's stationary e_b = [128, 2] one-hot pair writes its
            # point-sums into PSUM row b (the other row accumulates +0);
            # all 32 matmuls form ONE accumulation group at partition 0.
            MM_F = 2 * CH
            sred = pspool.tile([BL, MM_F], f32, tag="sred")
            nmm = BL * N_TILES * (TILE_F // MM_F)
            i = 0
            for b in range(BL):
                eb = wp16[:, OFF_EB + b:OFF_EB + b + 2]
                for t in range(N_TILES):
                    tin = iopool.tile([P, TILE_F], f16, tag="tin")
                    nc.sync.dma_start(out=tin[:, :], in_=lv[b * N_TILES + t, :, :])
                    for j in range(TILE_F // MM_F):
                        nc.tensor.matmul(sred[:, :], lhsT=eb,
                                         rhs=tin[:, j * MM_F:(j + 1) * MM_F],
                                         start=(i == 0), stop=(i == nmm - 1))
                        i += 1

            # fold [2, 512] -> fp16 [2, 256] sums via SBUF bounce (HW allows
            # only one PSUM input per TensorTensor)
            s512 = spool.tile([BL, MM_F], f32, tag="s512")
            nc.scalar.copy(out=s512[:, :], in_=sred[:, :])
            s16 = spool.tile([BL, CH], f16, tag="s16")
            nc.vector.tensor_add(out=s16[:, :], in0=s512[:, 0:CH],
                                 in1=s512[:, CH:MM_F])
            # transpose [2, 256] -> [128, 4] (cols b0k0 b1k0 b0k1 b1k1) via
            # identity-matmul; contraction over the 2 batch partitions.
            I2 = wp16[0:BL, OFF_I2:OFF_I2 + BL]
            mtp = mmpool.tile([P, 2 * BL], f32, tag="mm")
            for k in range(2):
                nc.tensor.matmul(mtp[:, k * BL:(k + 1) * BL],
                                 lhsT=s16[:, k * P:(k + 1) * P], rhs=I2,
                                 start=True, stop=True)
            # mean scale (1/4096, exact power of two) folded into the copy
            mt16 = spool.tile([P, 2 * BL], f16, tag="mt16")
            nc.scalar.activation(mt16[:, :], mtp[:, :], Act.Copy,
                                 scale=float(1.0 / NPTS))

            def leaky(zp, bias_col, tag):
                z = spool.tile([P, BL], f16, tag=f"z{tag}")
                nc.scalar.activation(z[:, :], zp[:, :], Act.Identity,
                                     bias=wp32[:, bias_col:bias_col + 1], scale=1.0)
                h = spool.tile([P, BL], f16, tag=f"h{tag}")
                nc.vector.scalar_tensor_tensor(out=h[:, :], in0=z[:, :], scalar=0.01,
                                               in1=z[:, :], op0=Alu.mult, op1=Alu.max)
                return h

            # h1 = leaky(m @ W1 + b1), W1 pre-folded on host; both batches at once
            h1p = mmpool.tile([P, BL], f32, tag="mm")
            for k in range(2):
                nc.tensor.matmul(h1p[:, :],
                                 lhsT=wp16[:, OFF_W1 + k * P: OFF_W1 + (k + 1) * P],
                                 rhs=mt16[:, k * BL:(k + 1) * BL],
                                 start=(k == 0), stop=(k == 1))
            h1 = leaky(h1p, C_B1, "1")

            h2p = mmpool.tile([P, BL], f32, tag="mm")
            nc.tensor.matmul(h2p[:, :], lhsT=wp16[:, OFF_WO2:OFF_WO2 + P],
                             rhs=h1[:, :], start=True, stop=True)
            h2 = leaky(h2p, C_B2, "2")

            # final layer in row form: h2^T @ wo3 -> [2, 256] (both batches)
            orp = mmpool.tile([BL, CH], f32, tag="mm")
            nc.tensor.matmul(orp[:, :], lhsT=h2[:, :],
                             rhs=wp16[:, OFF_WO3:OFF_WO3 + CH],
                             start=True, stop=True)
            orow = spool.tile([BL, CH], f32, tag="orow")
            nc.vector.tensor_add(out=orow[:, :], in0=orp[:, :], in1=b3row2[:, :])
            nc.scalar.dma_start(out=out_rows[:, :], in_=orow[:, :])

    nc.compile()
    return nc


def _pack_weights(inputs):
    wv = np.asarray(inputs["wv"], np.float64)
    wo1 = np.asarray(inputs["wo1"], np.float64)
    W1 = (wv @ wo1)                           # [256, 128], no nonlinearity between

    wp16 = np.zeros((P, W16_F), np.float16)
    wp16[:, OFF_W1:OFF_W1 + P] = W1[0:128, :]
    wp16[:, OFF_W1 + P:OFF_W1 + 2 * P] = W1[128:256, :]
    wp16[:, OFF_WO2:OFF_WO2 + P] = np.asarray(inputs["wo2"], np.float32)
    wp16[:, OFF_WO3:OFF_WO3 + CH] = np.asarray(inputs["wo3"], np.float32)
    wp16[:, OFF_EB + 0] = 1.0    # pair (640,641) = [1,0] -> row 0
    wp16[:, OFF_EB + 1] = 0.0    # pair (641,642) = [0,1] -> row 1
    wp16[:, OFF_EB + 2] = 1.0
    for b in range(BL):
        wp16[b, OFF_I2 + b] = 1.0   # [2, 2] identity for the transpose matmul

    wp32 = np.zeros((P, W32_F), np.float32)
    wp32[:, C_B1] = np.asarray(inputs["b1"], np.float32)
    wp32[:, C_B2] = np.asarray(inputs["b2"], np.float32)
    b3row2 = np.broadcast_to(
        np.asarray(inputs["b3"], np.float32).reshape(1, CH), (BL, CH)).copy()
    return wp16, wp32, b3row2


def kernel(**inputs):
    from concourse.bass_utils import run_bass_kernel_spmd

    if "nc" not in _CACHE:
        _CACHE["nc"] = _build_program()
    nc = _CACHE["nc"]

    lidar16 = np.ascontiguousarray(
        np.asarray(inputs["lidar"], dtype=np.float32).astype(np.float16))
    wp16, wp32, b3row2 = _pack_weights(inputs)

    in_maps = [
        {"lidar16": lidar16[i * BL:(i + 1) * BL], "wp16": wp16,
         "wp32": wp32, "b3row2": b3row2}
        for i in range(N_CORES)
    ]
    res = run_bass_kernel_spmd(nc, in_maps, list(range(N_CORES)),
                               **_CACHE.get("run_kwargs", {}))
    _CACHE["last_results"] = res
    out = np.concatenate([res.results[i]["out_rows"] for i in range(N_CORES)], axis=0)
    return np.ascontiguousarray(out, dtype=np.float32)
